# revision 13
# baseline (speedup 1.0000x reference)
"""Trainium2 Bass kernel for AttentiveTransformer (fc -> ghost BN ->
prior scaling -> sparsemax), data-parallel over 8 NeuronCores.

Per core (8192 rows), per 512-row macro tile, engine-balanced pipeline:
  - fp16 single-pass matmul into 2 PSUM groups (fp32 accumulate)
  - ghost-BN stats via per-chunk DVE bn_stats; bn_aggr/copies replaced
    by a batched even/odd-partial combine over all 8 chunks at once
    ([128,8]-wide GpSimd tensor_tensor ops reading the packed partials):
    128*var = (cve+cvo) + 32*(me-mo)^2, mean*2 = me+mo
  - BN apply: per-chunk, split DVE tensor_scalar (x*a+b, two scalar APs)
    for N_VA chunks / ACT Identity(scale,bias) for the rest, all into one
    fp16 xn tile; prior scaling = one GpSimd tensor_tensor per group
    (group 1 emitted first so its TT can start while group 0 applies run)
  - PE fp16 transposes into one fp16 PSUM bank
  - sparsemax top-8 tau: 4 DVE max8, zs-1 at segment heads, ONE segmented
    cumsum over [128,32] (scan with 0/1 restart mask), hneg = cssv*nrho,
    min-reduce; ACT Relu(z - tau) with per-row bias
  - fp16 output store (host upcasts); fh+priorsT packed into a single
    input DMA per macro, consts packed into 3 DMAs (Sync queue relief)
  - macro loop software-pipelined 4 deep; per-engine program order keeps
    ready work ahead of cross-engine round-trips
"""


import numpy as np
import concourse.bass as bass
import concourse.tile as tile
from concourse import bacc, mybir
from concourse.mybir import AluOpType as alu
from concourse.mybir import ActivationFunctionType as actf

F32 = mybir.dt.float32
F16 = mybir.dt.float16
IN, G = 512, 256
VBS = 128
EPS = 1e-5
MACRO = 512
# chunks applied on DVE via tensor_scalar (group 0, c < N_VA); rest on ACT
N_VA = 2
V_CHUNKS = [(0, c) for c in range(N_VA)]
S_CHUNKS = [(1, c) for c in range(4)] + [(0, c) for c in range(N_VA, 4)]


def build_program(bc: int, n_cores: int, repeat: int = 1):
    assert bc % MACRO == 0
    n_macro = bc // MACRO

    nc = bacc.Bacc(
        "TRN2",
        target_bir_lowering=False,
        debug=False,
        enable_asserts=False,
        num_devices=n_cores,
    )
    # fh rearranged [128, 4, bc] and priorsT rearranged [128, 2, bc],
    # packed along dim 1 -> one input DMA per macro
    fpin = nc.dram_tensor("fpin", [128, 6, bc], F16, kind="ExternalInput").ap()
    wh_p = nc.dram_tensor("wh_p", [128, 4, G], F16, kind="ExternalInput").ap()
    # consts blob: gam(8) bet(8) nrho(32) m1(32) mask01(32) = 112 floats
    cblob = nc.dram_tensor("cblob", [128, 112], F32, kind="ExternalInput").ap()
    ident = nc.dram_tensor("ident", [128, 128], F16, kind="ExternalInput").ap()
    out = nc.dram_tensor("out", [bc, G], F16, kind="ExternalOutput").ap()

    with tile.TileContext(nc) as tc:
        _body(tc, n_macro, fpin, wh_p, cblob, ident, out)
    nc.compile()
    return nc


def _body(tc, n_macro, fpin, wh_p, cblob, ident, out):
    nc = tc.nc
    with (
        tc.tile_pool(name="consts", bufs=1) as consts,
        tc.tile_pool(name="fp", bufs=6) as fpp,
        tc.tile_pool(name="bst", bufs=3) as bstp,
        tc.tile_pool(name="coef", bufs=3) as cfp,
        tc.tile_pool(name="xn_sb", bufs=3) as xnp,
        tc.tile_pool(name="z_sb", bufs=3) as zp,
        tc.tile_pool(name="topk", bufs=2) as tkp,
        tc.tile_pool(name="osb", bufs=3) as op_,
        tc.tile_pool(name="ps_xt", bufs=3, space="PSUM") as ps_xt,
        tc.tile_pool(name="ps_z", bufs=2, space="PSUM") as ps_z,
    ):
        st = {}

        def loads(t):
            fp = fpp.tile([128, 6, MACRO], F16, tag="fp")
            nc.sync.dma_start(
                fp[:], fpin[:, :, t * MACRO : (t + 1) * MACRO]
            )
            st[t] = {"fp": fp}

        loads(0)

        # ---- constants (3 DMAs) ----
        whb = consts.tile([128, 4, G], F16, tag="whb")
        nc.sync.dma_start(whb[:], wh_p)
        idn = consts.tile([128, 128], F16, tag="ident")
        nc.sync.dma_start(idn[:], ident)
        cb = consts.tile([128, 112], F32, tag="cb")
        nc.sync.dma_start(cb[:], cblob)
        gam = cb[:, 0:8]
        bet = cb[:, 8:16]
        nrho = cb[:, 16:48]
        m1 = cb[:, 48:80]
        msk = cb[:, 80:112]
        eps_t = consts.tile([128, 1], F32, tag="eps")
        nc.vector.memset(eps_t[:], EPS)
        c32 = consts.tile([128, 8], F32, tag="c32")
        nc.vector.memset(c32[:], 32.0)
        cm05 = consts.tile([128, 8], F32, tag="cm05")
        nc.vector.memset(cm05[:], -0.5)
        # touch the activation table early so ACT_TABLE_LOAD overlaps the
        # initial input DMA instead of stalling the first apply
        warm = consts.tile([128, 1], F32, tag="warm")
        nc.scalar.activation(warm[:], eps_t[:], actf.Square)

        # ---- per-macro stages ----
        def mm(t):
            xt_ps = []
            fp = st[t]["fp"]
            for g in range(2):
                xg = ps_xt.tile([128, MACRO], F32, tag=f"xt{g}")
                for k in range(4):
                    nc.tensor.matmul(
                        xg[:],
                        whb[:, k, g * 128 : (g + 1) * 128],
                        fp[:, k, :],
                        start=(k == 0),
                        stop=(k == 3),
                    )
                xt_ps.append(xg)
            st[t]["xt"] = xt_ps

        def stats(t):
            # per-chunk bn_stats; bst packs 8 chunks x 6 partials
            xt_ps = st[t]["xt"]
            bst = bstp.tile([128, 48], F32, tag="bst")
            for g in range(2):
                for c in range(4):
                    i = g * 4 + c
                    nc.vector.bn_stats(
                        bst[:, 6 * i : 6 * i + 6],
                        xt_ps[g][:, c * 128 : (c + 1) * 128],
                    )
            st[t]["bst"] = bst

        def coefA(t):
            # even/odd partial combine, all 8 chunks at once (GpSimd TTs):
            # 128*var = (cve + cvo) + 32*(me - mo)^2
            bstr = st[t]["bst"].rearrange("p (i s) -> p s i", s=6)
            me, mo = bstr[:, 1, :], bstr[:, 4, :]
            cve, cvo = bstr[:, 2, :], bstr[:, 5, :]
            vs = cfp.tile([128, 8], F32, tag="vs")
            nc.gpsimd.tensor_tensor(vs[:], cve, cvo, alu.add)
            md = cfp.tile([128, 8], F32, tag="md")
            nc.gpsimd.tensor_tensor(md[:], me, mo, alu.subtract)
            md32 = cfp.tile([128, 8], F32, tag="md32")
            nc.gpsimd.tensor_tensor(md32[:], md[:], c32[:], alu.mult)
            q32 = cfp.tile([128, 8], F32, tag="q32")
            nc.gpsimd.tensor_tensor(q32[:], md32[:], md[:], alu.mult)
            varsum = cfp.tile([128, 8], F32, tag="varsum")
            nc.gpsimd.tensor_tensor(varsum[:], vs[:], q32[:], alu.add)
            st[t]["varsum"] = varsum

        def sqA(t):
            varsum = st[t]["varsum"]
            sq = cfp.tile([128, 8], F32, tag="sq")
            nc.scalar.activation(
                sq[:], varsum[:], actf.Sqrt, bias=eps_t[:], scale=1.0 / 128.0
            )
            st[t]["sq"] = sq

        def rstdA(t):
            sq = st[t]["sq"]
            rstd = cfp.tile([128, 8], F32, tag="rstd")
            nc.vector.reciprocal(rstd[:], sq[:])
            st[t]["rstd"] = rstd

        def coefC(t):
            # a = gam * rstd ; b = bet - 0.5*(me+mo)*a
            rstd = st[t]["rstd"]
            bstr = st[t]["bst"].rearrange("p (i s) -> p s i", s=6)
            me, mo = bstr[:, 1, :], bstr[:, 4, :]
            a_t = cfp.tile([128, 8], F32, tag="a_t")
            nc.gpsimd.tensor_tensor(a_t[:], rstd[:], gam, alu.mult)
            ms = cfp.tile([128, 8], F32, tag="ms")
            nc.gpsimd.tensor_tensor(ms[:], me, mo, alu.add)
            msh = cfp.tile([128, 8], F32, tag="msh")
            nc.gpsimd.tensor_tensor(msh[:], ms[:], cm05[:], alu.mult)
            nm = cfp.tile([128, 8], F32, tag="nm")
            nc.gpsimd.tensor_tensor(nm[:], msh[:], a_t[:], alu.mult)
            b_t = cfp.tile([128, 8], F32, tag="b_t")
            nc.gpsimd.tensor_tensor(b_t[:], bet, nm[:], alu.add)
            st[t]["a"] = a_t
            st[t]["b"] = b_t

        def applyS(t):
            # ACT-route chunks: xn = a*x + b (allocates the shared xn tile)
            xt_ps = st[t]["xt"]
            a_t, b_t = st[t]["a"], st[t]["b"]
            xn = xnp.tile([128, 2, MACRO], F16, tag="xn")
            for g, c in S_CHUNKS:
                sl = slice(c * 128, (c + 1) * 128)
                i = g * 4 + c
                nc.scalar.activation(
                    xn[:, g, sl],
                    xt_ps[g][:, sl],
                    actf.Identity,
                    bias=b_t[:, i : i + 1],
                    scale=a_t[:, i : i + 1],
                )
            st[t]["xn"] = xn

        def applyV(t):
            # DVE-route chunks: xn = (x * a) + b via two-scalar tensor_scalar
            xt_ps = st[t]["xt"]
            a_t, b_t = st[t]["a"], st[t]["b"]
            xn = st[t]["xn"]
            for g, c in V_CHUNKS:
                sl = slice(c * 128, (c + 1) * 128)
                i = g * 4 + c
                nc.vector.tensor_scalar(
                    out=xn[:, g, sl],
                    in0=xt_ps[g][:, sl],
                    scalar1=a_t[:, i : i + 1],
                    scalar2=b_t[:, i : i + 1],
                    op0=alu.mult,
                    op1=alu.add,
                )

        def applyG(t, g):
            # prior scaling for one group: z_g = xn_g * p_g (one big TT)
            xn, fp = st[t]["xn"], st[t]["fp"]
            if "z" not in st[t]:
                st[t]["z"] = zp.tile([128, 2, MACRO], F16, tag="z", name="z")
            z = st[t]["z"]
            nc.gpsimd.tensor_tensor(
                z[:, g, :], xn[:, g, :], fp[:, 4 + g, :], alu.mult
            )

        def trans(t):
            z = st[t]["z"]
            zn = ps_z.tile([128, 4, 256], F16, tag="zn")
            for c in range(4):
                sl = slice(c * 128, (c + 1) * 128)
                nc.tensor.transpose(zn[:, c, 0:128], z[:, 0, sl], idn[:])
                nc.tensor.transpose(zn[:, c, 128:256], z[:, 1, sl], idn[:])
            st[t]["zn"] = zn

        def max8(t):
            zn = st[t]["zn"]
            zs = tkp.tile([128, 32], F32, tag="zs")
            for c in range(4):
                nc.vector.max(zs[:, c * 8 : c * 8 + 8], zn[:, c, :])
            st[t]["zs"] = zs

        def tau(t):
            zs = st[t]["zs"]
            zsm = tkp.tile([128, 32], F32, tag="zsm")
            nc.vector.tensor_tensor(zsm[:], zs[:], m1, alu.subtract)
            cssv = tkp.tile([128, 32], F32, tag="cssv")
            nc.vector.tensor_tensor_scan(
                cssv[:], msk, zsm[:], 0.0, alu.mult, alu.add
            )
            hneg = tkp.tile([128, 32], F32, tag="hneg")
            nc.vector.tensor_tensor(hneg[:], cssv[:], nrho, alu.mult)
            negtau = tkp.tile([128, 4], F32, tag="negtau")
            nc.vector.tensor_reduce(
                negtau[:],
                hneg[:].rearrange("p (c j) -> p c j", j=8),
                mybir.AxisListType.X,
                alu.min,
            )
            st[t]["negtau"] = negtau

        def relu(t):
            zn, negtau = st[t]["zn"], st[t]["negtau"]
            ob = op_.tile([128, 4, G], F16, tag="osb")
            for c in range(4):
                nc.scalar.activation(
                    ob[:, c, :], zn[:, c, :], actf.Relu,
                    bias=negtau[:, c : c + 1],
                )
            st[t]["ob"] = ob

        def store(t):
            ob = st[t]["ob"]
            nc.sync.dma_start(
                out[t * MACRO : (t + 1) * MACRO, :].rearrange(
                    "(c p) g -> p c g", p=128
                ),
                ob[:],
            )
            del st[t]

        def coef_chain(t):
            coefA(t)
            sqA(t)
            rstdA(t)
            coefC(t)

        # ---- prologue ----
        loads(1)
        loads(2)
        loads(3)
        mm(0)
        stats(0)
        mm(1)
        stats(1)
        coef_chain(0)
        applyS(0)
        applyV(0)
        applyG(0, 1)
        applyG(0, 0)
        trans(0)
        mm(2)
        stats(2)
        coef_chain(1)

        # ---- steady-state loop ----
        # per-engine order per iteration:
        #   DVE:    ts-apply x2 (t+1), max8 x4 (t), zsm/scan/hneg/reduce (t),
        #           rstd (t+2), bn_stats x8 (t+3)
        #   Scalar: ACT-apply x6 (t+1; group 1 first), sq (t+2), relu x4 (t)
        #   GpSimd: coefA x5 (t+2), prior-TT g1 (t+1), prior-TT g0 (t+1),
        #           coefC x5 (t+2)
        #   PE:     mm (t+3), transposes (t+1)
        for t in range(n_macro):
            if t + 4 < n_macro:
                loads(t + 4)
            if t + 3 < n_macro:
                mm(t + 3)
            if t + 2 < n_macro:
                coefA(t + 2)
            if t + 1 < n_macro:
                applyS(t + 1)
                applyV(t + 1)
                applyG(t + 1, 1)
            max8(t)
            if t + 2 < n_macro:
                sqA(t + 2)
            tau(t)
            if t + 2 < n_macro:
                rstdA(t + 2)
            if t + 1 < n_macro:
                applyG(t + 1, 0)
            if t + 2 < n_macro:
                coefC(t + 2)
            relu(t)
            if t + 1 < n_macro:
                trans(t + 1)
            store(t)
            if t + 3 < n_macro:
                stats(t + 3)


def host_prep(priors, processed_feat, W, gamma, beta, n_cores):
    B = priors.shape[0]
    bc = B // n_cores
    Wf = W.astype(np.float32)
    wTh = Wf.T.astype(np.float16)            # [IN, G]
    wh_p = np.ascontiguousarray(wTh.reshape(4, 128, G).transpose(1, 0, 2))
    g8 = np.tile(gamma.astype(np.float32).reshape(2, 128).T[:, :, None], (1, 1, 4))
    gam8 = g8.reshape(128, 8)
    b8 = np.tile(beta.astype(np.float32).reshape(2, 128).T[:, :, None], (1, 1, 4))
    bet8 = b8.reshape(128, 8)
    nrhoinv = np.tile(
        (-1.0 / np.arange(1, 9, dtype=np.float32)), (128, 4)
    ).astype(np.float32)
    m1row = np.zeros(32, dtype=np.float32)
    m1row[::8] = 1.0
    m1c = np.tile(m1row, (128, 1)).astype(np.float32)
    mask01 = (1.0 - m1c).astype(np.float32)
    cblob = np.ascontiguousarray(
        np.concatenate([gam8, bet8, nrhoinv, m1c, mask01], axis=1)
    ).astype(np.float32)
    ident = np.eye(128, dtype=np.float16)
    in_maps = []
    for i in range(n_cores):
        sl = slice(i * bc, (i + 1) * bc)
        feat_s = processed_feat[sl].astype(np.float32)
        fhi = feat_s.T.astype(np.float16)                      # [IN, bc]
        pTi = priors[sl].astype(np.float16).T                  # [G, bc]
        fpin = np.concatenate(
            [
                fhi.reshape(4, 128, bc).transpose(1, 0, 2),
                pTi.reshape(2, 128, bc).transpose(1, 0, 2),
            ],
            axis=1,
        )
        in_maps.append(
            {
                "fpin": np.ascontiguousarray(fpin),
                "wh_p": wh_p,
                "cblob": cblob,
                "ident": ident,
            }
        )
    return in_maps


# ---------------------------------------------------------------------------
# Harness entry point
# ---------------------------------------------------------------------------

N_CORES = 8
_PROGRAM_CACHE = {}


def _get_program(bc):
    if bc not in _PROGRAM_CACHE:
        _PROGRAM_CACHE[bc] = build_program(bc, N_CORES)
    return _PROGRAM_CACHE[bc]


def kernel(priors, processed_feat, W, gamma, beta):
    """Full-input entry: shards the batch over 8 NeuronCores, runs the
    Bass kernel, gathers the full [B, G] float32 output."""
    from concourse.bass_utils import run_bass_kernel_spmd

    priors = np.asarray(priors)
    processed_feat = np.asarray(processed_feat)
    W = np.asarray(W)
    gamma = np.asarray(gamma)
    beta = np.asarray(beta)
    B = priors.shape[0]
    bc = B // N_CORES
    assert B % N_CORES == 0 and bc % MACRO == 0, f"unsupported batch {B}"

    nc = _get_program(bc)
    in_maps = host_prep(priors, processed_feat, W, gamma, beta, N_CORES)
    last_err = None
    for attempt in range(3):
        try:
            res = run_bass_kernel_spmd(nc, in_maps, core_ids=list(range(N_CORES)))
            break
        except Exception as e:  # transient device/terminal flakes
            last_err = e
            import time as _time

            _time.sleep(10 * (attempt + 1))
    else:
        raise last_err
    out = np.concatenate([res.results[c]["out"] for c in range(N_CORES)], axis=0)
    return out.astype(np.float32)


# revision 15
# speedup vs baseline: 1.0682x; 1.0682x over previous
"""Trainium2 Bass kernel for AttentiveTransformer (fc -> ghost BN ->
prior scaling -> sparsemax), data-parallel over 8 NeuronCores.

Per core (8192 rows), per 512-row macro tile, engine-balanced pipeline:
  - fp16 single-pass matmul into 2 PSUM groups (fp32 accumulate)
  - ghost-BN stats via per-chunk DVE bn_stats; bn_aggr/copies replaced
    by a batched even/odd-partial combine over all 8 chunks at once
    ([128,8]-wide GpSimd tensor_tensor ops reading the packed partials):
    128*var = (cve+cvo) + 32*(me-mo)^2, mean*2 = me+mo
  - BN apply: per-chunk, split DVE tensor_scalar (x*a+b, two scalar APs)
    for N_VA chunks / ACT Identity(scale,bias) for the rest, all into one
    fp16 xn tile; prior scaling = one GpSimd tensor_tensor per group
    (group 1 emitted first so its TT can start while group 0 applies run)
  - PE fp16 transposes into one fp16 PSUM bank
  - sparsemax top-8 tau: 4 DVE max8, zs-1 at segment heads, ONE segmented
    cumsum over [128,32] (scan with 0/1 restart mask), hneg = cssv*nrho,
    min-reduce; ACT Relu(z - tau) with per-row bias
  - fp16 output store (host upcasts); fh+priorsT packed into a single
    input DMA per macro, consts packed into 3 DMAs (Sync queue relief)
  - macro loop software-pipelined 4 deep; per-engine program order keeps
    ready work ahead of cross-engine round-trips
"""


import numpy as np
import concourse.bass as bass
import concourse.tile as tile
from concourse import bacc, mybir
from concourse.mybir import AluOpType as alu
from concourse.mybir import ActivationFunctionType as actf

F32 = mybir.dt.float32
F16 = mybir.dt.float16
IN, G = 512, 256
VBS = 128
EPS = 1e-5
MACRO = 512
# chunks applied on DVE via tensor_scalar (group 0, c < N_VA); rest on ACT
N_VA = 2
V_CHUNKS = [(0, c) for c in range(N_VA)]
S_CHUNKS = [(0, c) for c in range(N_VA, 4)] + [(1, c) for c in range(4)]


def build_program(bc: int, n_cores: int, repeat: int = 1):
    assert bc % MACRO == 0
    n_macro = bc // MACRO

    nc = bacc.Bacc(
        "TRN2",
        target_bir_lowering=False,
        debug=False,
        enable_asserts=False,
        num_devices=n_cores,
    )
    # fh rearranged [128, 4, bc] and priorsT rearranged [128, 2, bc],
    # packed along dim 1 -> one input DMA per macro
    fpin = nc.dram_tensor("fpin", [128, 6, bc], F16, kind="ExternalInput").ap()
    wh_p = nc.dram_tensor("wh_p", [128, 4, G], F16, kind="ExternalInput").ap()
    # consts blob: gam(8) bet(8) nrho(32) m1(32) mask01(32) = 112 floats
    cblob = nc.dram_tensor("cblob", [128, 112], F32, kind="ExternalInput").ap()
    ident = nc.dram_tensor("ident", [128, 128], F16, kind="ExternalInput").ap()
    out = nc.dram_tensor("out", [bc, G], F16, kind="ExternalOutput").ap()

    with tile.TileContext(nc) as tc:
        _body(tc, n_macro, fpin, wh_p, cblob, ident, out)
    nc.compile()
    return nc


def _body(tc, n_macro, fpin, wh_p, cblob, ident, out):
    nc = tc.nc
    with (
        tc.tile_pool(name="consts", bufs=1) as consts,
        tc.tile_pool(name="fp", bufs=6) as fpp,
        tc.tile_pool(name="bst", bufs=3) as bstp,
        tc.tile_pool(name="coef", bufs=3) as cfp,
        tc.tile_pool(name="xn_sb", bufs=3) as xnp,
        tc.tile_pool(name="z_sb", bufs=3) as zp,
        tc.tile_pool(name="topk", bufs=2) as tkp,
        tc.tile_pool(name="osb", bufs=3) as op_,
        tc.tile_pool(name="ps_xt", bufs=3, space="PSUM") as ps_xt,
        tc.tile_pool(name="ps_z", bufs=2, space="PSUM") as ps_z,
    ):
        st = {}

        def loads(t):
            fp = fpp.tile([128, 6, MACRO], F16, tag="fp")
            nc.sync.dma_start(
                fp[:], fpin[:, :, t * MACRO : (t + 1) * MACRO]
            )
            st[t] = {"fp": fp}

        loads(0)

        # ---- constants (3 DMAs) ----
        whb = consts.tile([128, 4, G], F16, tag="whb")
        nc.sync.dma_start(whb[:], wh_p)
        idn = consts.tile([128, 128], F16, tag="ident")
        nc.sync.dma_start(idn[:], ident)
        cb = consts.tile([128, 112], F32, tag="cb")
        nc.sync.dma_start(cb[:], cblob)
        gam = cb[:, 0:8]
        bet = cb[:, 8:16]
        nrho = cb[:, 16:48]
        m1 = cb[:, 48:80]
        msk = cb[:, 80:112]
        eps_t = consts.tile([128, 1], F32, tag="eps")
        nc.vector.memset(eps_t[:], EPS)
        c32 = consts.tile([128, 8], F32, tag="c32")
        nc.vector.memset(c32[:], 32.0)
        cm05 = consts.tile([128, 8], F32, tag="cm05")
        nc.vector.memset(cm05[:], -0.5)
        # touch the activation table early so ACT_TABLE_LOAD overlaps the
        # initial input DMA instead of stalling the first apply
        warm = consts.tile([128, 1], F32, tag="warm")
        nc.scalar.activation(warm[:], eps_t[:], actf.Square)

        # ---- per-macro stages ----
        def mm(t):
            xt_ps = []
            fp = st[t]["fp"]
            for g in range(2):
                xg = ps_xt.tile([128, MACRO], F32, tag=f"xt{g}")
                for k in range(4):
                    nc.tensor.matmul(
                        xg[:],
                        whb[:, k, g * 128 : (g + 1) * 128],
                        fp[:, k, :],
                        start=(k == 0),
                        stop=(k == 3),
                    )
                xt_ps.append(xg)
            st[t]["xt"] = xt_ps

        def stats(t):
            # per-chunk bn_stats; bst packs 8 chunks x 6 partials
            xt_ps = st[t]["xt"]
            bst = bstp.tile([128, 48], F32, tag="bst")
            for g in range(2):
                for c in range(4):
                    i = g * 4 + c
                    nc.vector.bn_stats(
                        bst[:, 6 * i : 6 * i + 6],
                        xt_ps[g][:, c * 128 : (c + 1) * 128],
                    )
            st[t]["bst"] = bst

        def coefA(t):
            # even/odd partial combine, all 8 chunks at once (GpSimd TTs):
            # 128*var = (cve + cvo) + 32*(me - mo)^2
            bstr = st[t]["bst"].rearrange("p (i s) -> p s i", s=6)
            me, mo = bstr[:, 1, :], bstr[:, 4, :]
            cve, cvo = bstr[:, 2, :], bstr[:, 5, :]
            vs = cfp.tile([128, 8], F32, tag="vs")
            nc.gpsimd.tensor_tensor(vs[:], cve, cvo, alu.add)
            md = cfp.tile([128, 8], F32, tag="md")
            nc.gpsimd.tensor_tensor(md[:], me, mo, alu.subtract)
            md32 = cfp.tile([128, 8], F32, tag="md32")
            nc.gpsimd.tensor_tensor(md32[:], md[:], c32[:], alu.mult)
            q32 = cfp.tile([128, 8], F32, tag="q32")
            nc.gpsimd.tensor_tensor(q32[:], md32[:], md[:], alu.mult)
            varsum = cfp.tile([128, 8], F32, tag="varsum")
            nc.gpsimd.tensor_tensor(varsum[:], vs[:], q32[:], alu.add)
            st[t]["varsum"] = varsum

        def sqA(t):
            varsum = st[t]["varsum"]
            sq = cfp.tile([128, 8], F32, tag="sq")
            nc.scalar.activation(
                sq[:], varsum[:], actf.Sqrt, bias=eps_t[:], scale=1.0 / 128.0
            )
            st[t]["sq"] = sq

        def rstdA(t):
            sq = st[t]["sq"]
            rstd = cfp.tile([128, 8], F32, tag="rstd")
            nc.vector.reciprocal(rstd[:], sq[:])
            st[t]["rstd"] = rstd

        def coefC(t):
            # a = gam * rstd ; b = bet - 0.5*(me+mo)*a
            rstd = st[t]["rstd"]
            bstr = st[t]["bst"].rearrange("p (i s) -> p s i", s=6)
            me, mo = bstr[:, 1, :], bstr[:, 4, :]
            a_t = cfp.tile([128, 8], F32, tag="a_t")
            nc.gpsimd.tensor_tensor(a_t[:], rstd[:], gam, alu.mult)
            ms = cfp.tile([128, 8], F32, tag="ms")
            nc.gpsimd.tensor_tensor(ms[:], me, mo, alu.add)
            msh = cfp.tile([128, 8], F32, tag="msh")
            nc.gpsimd.tensor_tensor(msh[:], ms[:], cm05[:], alu.mult)
            nm = cfp.tile([128, 8], F32, tag="nm")
            nc.gpsimd.tensor_tensor(nm[:], msh[:], a_t[:], alu.mult)
            b_t = cfp.tile([128, 8], F32, tag="b_t")
            nc.gpsimd.tensor_tensor(b_t[:], bet, nm[:], alu.add)
            st[t]["a"] = a_t
            st[t]["b"] = b_t

        def applyS(t):
            # ACT-route chunks: xn = a*x + b (allocates the shared xn tile)
            xt_ps = st[t]["xt"]
            a_t, b_t = st[t]["a"], st[t]["b"]
            xn = xnp.tile([128, 2, MACRO], F16, tag="xn")
            for g, c in S_CHUNKS:
                sl = slice(c * 128, (c + 1) * 128)
                i = g * 4 + c
                nc.scalar.activation(
                    xn[:, g, sl],
                    xt_ps[g][:, sl],
                    actf.Identity,
                    bias=b_t[:, i : i + 1],
                    scale=a_t[:, i : i + 1],
                )
            st[t]["xn"] = xn

        def applyV(t):
            # DVE-route chunks: xn = (x * a) + b via two-scalar tensor_scalar
            xt_ps = st[t]["xt"]
            a_t, b_t = st[t]["a"], st[t]["b"]
            xn = st[t]["xn"]
            for g, c in V_CHUNKS:
                sl = slice(c * 128, (c + 1) * 128)
                i = g * 4 + c
                nc.vector.tensor_scalar(
                    out=xn[:, g, sl],
                    in0=xt_ps[g][:, sl],
                    scalar1=a_t[:, i : i + 1],
                    scalar2=b_t[:, i : i + 1],
                    op0=alu.mult,
                    op1=alu.add,
                )

        def applyG(t, g):
            # prior scaling for one group: z_g = xn_g * p_g (one big TT)
            xn, fp = st[t]["xn"], st[t]["fp"]
            if "z" not in st[t]:
                st[t]["z"] = zp.tile([128, 2, MACRO], F16, tag="z", name="z")
            z = st[t]["z"]
            nc.gpsimd.tensor_tensor(
                z[:, g, :], xn[:, g, :], fp[:, 4 + g, :], alu.mult
            )

        def trans(t):
            z = st[t]["z"]
            zn = ps_z.tile([128, 4, 256], F16, tag="zn")
            for c in range(4):
                sl = slice(c * 128, (c + 1) * 128)
                nc.tensor.transpose(zn[:, c, 0:128], z[:, 0, sl], idn[:])
                nc.tensor.transpose(zn[:, c, 128:256], z[:, 1, sl], idn[:])
            st[t]["zn"] = zn

        def max8(t):
            zn = st[t]["zn"]
            zs = tkp.tile([128, 32], F32, tag="zs")
            for c in range(4):
                nc.vector.max(zs[:, c * 8 : c * 8 + 8], zn[:, c, :])
            st[t]["zs"] = zs

        def tau(t):
            zs = st[t]["zs"]
            zsm = tkp.tile([128, 32], F32, tag="zsm")
            nc.vector.tensor_tensor(zsm[:], zs[:], m1, alu.subtract)
            cssv = tkp.tile([128, 32], F32, tag="cssv")
            nc.vector.tensor_tensor_scan(
                cssv[:], msk, zsm[:], 0.0, alu.mult, alu.add
            )
            hneg = tkp.tile([128, 32], F32, tag="hneg")
            nc.vector.tensor_tensor(hneg[:], cssv[:], nrho, alu.mult)
            negtau = tkp.tile([128, 4], F32, tag="negtau")
            nc.vector.tensor_reduce(
                negtau[:],
                hneg[:].rearrange("p (c j) -> p c j", j=8),
                mybir.AxisListType.X,
                alu.min,
            )
            st[t]["negtau"] = negtau

        def relu(t):
            zn, negtau = st[t]["zn"], st[t]["negtau"]
            ob = op_.tile([128, 4, G], F16, tag="osb")
            for c in range(4):
                nc.scalar.activation(
                    ob[:, c, :], zn[:, c, :], actf.Relu,
                    bias=negtau[:, c : c + 1],
                )
            st[t]["ob"] = ob

        def store(t):
            ob = st[t]["ob"]
            nc.sync.dma_start(
                out[t * MACRO : (t + 1) * MACRO, :].rearrange(
                    "(c p) g -> p c g", p=128
                ),
                ob[:],
            )
            del st[t]

        def coef_chain(t):
            coefA(t)
            sqA(t)
            rstdA(t)
            coefC(t)

        # ---- prologue ----
        loads(1)
        loads(2)
        loads(3)
        mm(0)
        stats(0)
        mm(1)
        stats(1)
        coef_chain(0)
        applyS(0)
        applyV(0)
        applyG(0, 0)
        applyG(0, 1)
        trans(0)
        mm(2)
        stats(2)
        coef_chain(1)

        # ---- steady-state loop ----
        # per-engine order per iteration (it-3 ordering):
        #   DVE:    max8 x4 (t), ts-apply x2 (t+1), zsm/scan/hneg/reduce (t),
        #           rstd (t+2), bn_stats x8 (t+3)
        #   Scalar: ACT-apply x6 (t+1), sq (t+2), relu x4 (t)
        #   GpSimd: coefA x5 (t+2), prior-TT g0 (t+1), coefC x5 (t+2),
        #           prior-TT g1 (t+1)
        #   PE:     mm (t+3), transposes (t+1)
        for t in range(n_macro):
            if t + 4 < n_macro:
                loads(t + 4)
            if t + 3 < n_macro:
                mm(t + 3)
            if t + 2 < n_macro:
                coefA(t + 2)
            max8(t)
            if t + 1 < n_macro:
                applyS(t + 1)
                applyV(t + 1)
                applyG(t + 1, 0)
            if t + 2 < n_macro:
                sqA(t + 2)
            tau(t)
            if t + 2 < n_macro:
                rstdA(t + 2)
                coefC(t + 2)
            if t + 1 < n_macro:
                applyG(t + 1, 1)
            relu(t)
            if t + 1 < n_macro:
                trans(t + 1)
            store(t)
            if t + 3 < n_macro:
                stats(t + 3)


def host_prep(priors, processed_feat, W, gamma, beta, n_cores):
    B = priors.shape[0]
    bc = B // n_cores
    Wf = W.astype(np.float32)
    wTh = Wf.T.astype(np.float16)            # [IN, G]
    wh_p = np.ascontiguousarray(wTh.reshape(4, 128, G).transpose(1, 0, 2))
    g8 = np.tile(gamma.astype(np.float32).reshape(2, 128).T[:, :, None], (1, 1, 4))
    gam8 = g8.reshape(128, 8)
    b8 = np.tile(beta.astype(np.float32).reshape(2, 128).T[:, :, None], (1, 1, 4))
    bet8 = b8.reshape(128, 8)
    nrhoinv = np.tile(
        (-1.0 / np.arange(1, 9, dtype=np.float32)), (128, 4)
    ).astype(np.float32)
    m1row = np.zeros(32, dtype=np.float32)
    m1row[::8] = 1.0
    m1c = np.tile(m1row, (128, 1)).astype(np.float32)
    mask01 = (1.0 - m1c).astype(np.float32)
    cblob = np.ascontiguousarray(
        np.concatenate([gam8, bet8, nrhoinv, m1c, mask01], axis=1)
    ).astype(np.float32)
    ident = np.eye(128, dtype=np.float16)
    in_maps = []
    for i in range(n_cores):
        sl = slice(i * bc, (i + 1) * bc)
        feat_s = processed_feat[sl].astype(np.float32)
        fhi = feat_s.T.astype(np.float16)                      # [IN, bc]
        pTi = priors[sl].astype(np.float16).T                  # [G, bc]
        fpin = np.concatenate(
            [
                fhi.reshape(4, 128, bc).transpose(1, 0, 2),
                pTi.reshape(2, 128, bc).transpose(1, 0, 2),
            ],
            axis=1,
        )
        in_maps.append(
            {
                "fpin": np.ascontiguousarray(fpin),
                "wh_p": wh_p,
                "cblob": cblob,
                "ident": ident,
            }
        )
    return in_maps


# ---------------------------------------------------------------------------
# Harness entry point
# ---------------------------------------------------------------------------

N_CORES = 8
_PROGRAM_CACHE = {}


def _get_program(bc):
    if bc not in _PROGRAM_CACHE:
        _PROGRAM_CACHE[bc] = build_program(bc, N_CORES)
    return _PROGRAM_CACHE[bc]


def kernel(priors, processed_feat, W, gamma, beta):
    """Full-input entry: shards the batch over 8 NeuronCores, runs the
    Bass kernel, gathers the full [B, G] float32 output."""
    from concourse.bass_utils import run_bass_kernel_spmd

    priors = np.asarray(priors)
    processed_feat = np.asarray(processed_feat)
    W = np.asarray(W)
    gamma = np.asarray(gamma)
    beta = np.asarray(beta)
    B = priors.shape[0]
    bc = B // N_CORES
    assert B % N_CORES == 0 and bc % MACRO == 0, f"unsupported batch {B}"

    nc = _get_program(bc)
    in_maps = host_prep(priors, processed_feat, W, gamma, beta, N_CORES)
    last_err = None
    for attempt in range(3):
        try:
            res = run_bass_kernel_spmd(nc, in_maps, core_ids=list(range(N_CORES)))
            break
        except Exception as e:  # transient device/terminal flakes
            last_err = e
            import time as _time

            _time.sleep(10 * (attempt + 1))
    else:
        raise last_err
    out = np.concatenate([res.results[c]["out"] for c in range(N_CORES)], axis=0)
    return out.astype(np.float32)
